# revision 47
# baseline (speedup 1.0000x reference)
"""Bior 2x upsampling (zero-interleave + separable 9-tap filter) on 8 TRN2 cores.

Math: y[n] = sum_m h[n+4-2m] x[m] along each spatial axis (SAME zero padding).
Both separable stages are banded matmuls on the TensorEngine:

  stage 1: T1[w, nh] = sum_h X[h, w]  * A[nh, h]   (lhsT = X,  K = h)
  stage 2: Y[nh, nw] = sum_w T1[w, nh] * A[nw, w]  (lhsT = T1, K = w)

with A[n, m] = h[n+4-2m]. Because A is shift invariant, every matmul's rhs
is a column-slice of constant matrices CAx[i,j] = h[j+4-2i] /
CBx[i,j] = h[j-256-2i] ([128, 520] each, shipped once).

Everything is bf16 except PSUM accumulation (f32): input shipped bf16
(1 MiB/core), output stored bf16 (4 MiB/core) and upcast to f32 on the
host - halves HBM traffic vs f32 at ~7.6e-3 rel err (tolerance 2e-2).
bf16 matmuls also avoid the f32r N<256 cycle penalty.

Default SCHEME "corner4": K-tiles 128-aligned in both stages, so ONE
input DMA burst loads x (avoids HBM read/write turnaround); each
128x1024 block is FOUR matmuls - each interior K-tile's full 263-col
contribution is one contiguous cbx band that CROSSES the 512-col PSUM
bank boundary (legal on this toolchain). 1046 streamed cols/block vs
1560 for the classic 6-MM main+corner split ("corner"), cutting PE busy
15.6->10.5us/rep. Ordering constraint: the two non-crossing MMs carry
start=True and clear each bank's has_written bits FIRST; the crossing
MMs then accumulate on overlaps / overwrite elsewhere.
Alternatives kept for reference: "corner" (6 uniform MMs), and
"banded5"/"hybrid" (K-tiles at B5_OFF with 4-row overlaps so every band
is disjoint - but +2 evac tiles/rep and 4 input DMAs make them slower;
evac, not PE, is the tighter resource).

Pipelining (the difference between 37us and ~16us at identical work):
 - PE p-state: the tensor engine only reaches 2.4GHz after ~3us of
   gap-free execution (else 1.2GHz). All structure below serves PE
   density: stage 1 of BOTH images runs before any stage 2 (evac runway),
   4 narrow [128,1024] PSUM bufs give 3-group rotation slack (~2us).
 - PSUM evacuation (PSUM->SBUF copies; DMA cannot read PSUM) is the
   second-largest engine cost (~26624 cols/rep). Whole-tile copies
   alternate ACT/DVE on a Bresenham pattern (EVAC_RATIO on ACT, which is
   ~1.25x faster/elem); per-tile splitting costs more in fixed
   PSUM-access overhead (~185ns/instr) than it saves.
 - input DMA triggers ride the otherwise-idle SP(sync) ring - on ACT
   they queue behind a whole rep of evacs and serialize the input.
 - timing loop: tile pools only rotate per tile() call at trace time, so
   UNROLL reps are emitted per For_i body for cross-rep double-buffering,
   and each body's x buffer is refilled MID-body (loop data is identical
   every rep) giving ~1.5 reps of prefetch distance.

Sharding: pure data parallel, 2 images per core across 8 cores.
Engine floors per rep (2 images): PE 10.5us, ACT/DVE evac ~14/12us at
EVAC_RATIO 0.58, DMA 13.7-15.3us (5.25 MiB at ~360-400GB/s) ->
measured ~15.2-16.6us (was 37.4us baseline).
"""

import numpy as np

H_TILDE = np.array([0.03782845550699535, -0.02384946501937986, -0.1106244044184226,
                    0.3774028556126536, 0.8526986790094022, 0.3774028556126537,
                    -0.1106244044184226, -0.02384946501937986, 0.03782845550699535],
                   dtype=np.float32)

B_PER_CORE = 2
N_CORES = 8
H = W = 512
HO = WO = 1024

# "bf16" (2x PE rate + half DMA bytes, ~5e-3 rel err), "f32r" (~2e-4),
# or "f32" (~1e-6, 8x slower matmuls)
MM_DTYPE = "bf16"
# y stored in HBM as bf16 (half the output write traffic), upcast on host
OUT_DTYPE = "bf16"
EVAC_MODE = "ratio"
# "corner": aligned 128-row K-tiles + full-width corner MMs (6 MMs, 1560
# cols per 128x1024 block). "banded5": K-tiles at stride 124 (4-row
# overlap, re-reads 16 rows/image of x) so each K-tile contributes one
# disjoint output band - 1024 cols per block, no corners. "hybrid":
# corner structure for stage 1 (keeps the single aligned input DMA) but
# its 5 M-slices sit at B5_OFF so stage 2 runs banded - 1024 cols/block.
# corner measured fastest: banded/hybrid trade PE cols for 2 extra evac
# tiles/rep, and evac is the tighter resource.
SCHEME = "corner4"
# fraction of evac tiles copied by ACT (rest DVE); with corner4's light
# PE the evac engines bind, and ACT-heavy measures best
EVAC_RATIO = 0.58
OUT_RING = "sync"
IN_ENG = "sync"
STAGGERED = False
MM_ORDER = "banks"
# reps unrolled inside the For_i timing body: tile pools only rotate per
# tile() call at trace time, so cross-iteration double-buffering (x
# prefetch, y drain) needs >=2 reps in the traced body
UNROLL = 4
PSP_BUFS = 4
XP_BUFS = 2
T1P_BUFS = 12
YP_BUFS = 4
# output rows per DMA: 4 -> four 1MiB writes/rep instead of eight 512KiB
# (fewer DMA instruction overheads + write-burst turnarounds, ~1.2us)
Y_GROUP = 4

_CACHE = {}


def _consts():
    """One [128, 1040] f32 constant: CAx | CBx (each [128, 520]).

    CAx[i, j] = h[j + 4 - 2i], CBx[i, j] = h[j - 256 - 2i]. Slices:
      main  rhs aligned at +0   : cax[:, 0:260]   /  cbx[:, 256:516]
      corner rhs (same N=260)   : cax[:, 256:516] /  cbx[:, 0:260]
    """
    h = H_TILDE
    cax = np.zeros((128, 520), dtype=np.float32)
    cbx = np.zeros((128, 520), dtype=np.float32)
    for i in range(128):
        for j in range(520):
            k = j + 4 - 2 * i
            if 0 <= k <= 8:
                cax[i, j] = h[k]
            k = j - 256 - 2 * i
            if 0 <= k <= 8:
                cbx[i, j] = h[k]
    return np.concatenate([cax, cbx], axis=1)


def _split_multiwaits(nc, mybir):
    """walrus here encodes at most ONE sem-wait per instruction; hoist extras
    onto preceding same-engine nops (sequencer order => identical semantics)."""
    ctr = 0
    for fn in nc.m.functions:
        for bb in fn.blocks:
            out, changed = [], False
            for ins in bb.instructions:
                si = ins.sync_info
                if si is not None and len(si.on_wait) > 1:
                    waits = list(si.on_wait)
                    for w in waits[:-1]:
                        ctr += 1
                        nop = mybir.InstNoOp(name=f"wsplit-{ctr}", ins=[], outs=[])
                        nop.engine = ins.engine
                        nop.sync_info = mybir.SyncInfo(on_wait=[w], on_update=[])
                        out.append(nop)
                    si.on_wait = [waits[-1]]
                    changed = True
                out.append(ins)
            if changed:
                bb.instructions = out
    return ctr


def _emit_block(nc, ps, src, mlo, mhi, cax, cbx, f32r, MM_ORDER=None):
    if MM_ORDER is None:
        MM_ORDER = globals()["MM_ORDER"]
    """Emit the 6 uniform [K=128, M=128, N=260] matmuls for one block.

    ps: PSUM [128, 1024]; src: 4 source tiles (partitions = contraction dim);
    mlo:mhi: the 128-wide free-dim slice of the source tiles forming M.
    Corners are full-shape MMs whose rhs is mostly zeros (uniform shape
    keeps the PE pipeline dense; tiny-N MMs measured ~600ns each)."""
    mm = nc.tensor.matmul
    kw = dict(skip_group_check=True)
    if MM_ORDER == "banks":
        mm(ps[:, 0:260], lhsT=src[0][:, mlo:mhi], rhs=cax[:, 0:260],
           start=True, stop=False, **kw)
        mm(ps[:, 252:512], lhsT=src[1][:, mlo:mhi], rhs=cbx[:, 256:516],
           start=False, stop=False, **kw)
        mm(ps[:, 252:512], lhsT=src[2][:, mlo:mhi], rhs=cbx[:, 0:260],
           start=False, stop=False, **kw)
        mm(ps[:, 512:772], lhsT=src[2][:, mlo:mhi], rhs=cax[:, 0:260],
           start=True, stop=False, **kw)
        mm(ps[:, 512:772], lhsT=src[1][:, mlo:mhi], rhs=cax[:, 256:516],
           start=False, stop=False, **kw)
        mm(ps[:, 764:1024], lhsT=src[3][:, mlo:mhi], rhs=cbx[:, 256:516],
           start=False, stop=True, **kw)
    else:  # "paired": same-lhsT MMs adjacent; bank1's first writer is the
           # tile1 corner (start=True overwrites with zeros+corner, then
           # tile2 main accumulates) — identical math via has_written rules
        mm(ps[:, 0:260], lhsT=src[0][:, mlo:mhi], rhs=cax[:, 0:260],
           start=True, stop=False, **kw)
        mm(ps[:, 252:512], lhsT=src[1][:, mlo:mhi], rhs=cbx[:, 256:516],
           start=False, stop=False, **kw)
        mm(ps[:, 512:772], lhsT=src[1][:, mlo:mhi], rhs=cax[:, 256:516],
           start=True, stop=False, **kw)
        mm(ps[:, 252:512], lhsT=src[2][:, mlo:mhi], rhs=cbx[:, 0:260],
           start=False, stop=False, **kw)
        mm(ps[:, 512:772], lhsT=src[2][:, mlo:mhi], rhs=cax[:, 0:260],
           start=False, stop=False, **kw)
        mm(ps[:, 764:1024], lhsT=src[3][:, mlo:mhi], rhs=cbx[:, 256:516],
           start=False, stop=True, **kw)


# banded5: K-tile row offsets (4-row overlap at stride 124; tiles 0 and 4
# aligned so all reads stay in [0, 512)) and the disjoint output band each
# K-tile produces: (out_lo, out_hi, ktile, rhs col offset into cax).
# A band crossing the 512-col PSUM bank boundary is split at 512.
B5_OFF = [0, 122, 246, 370, 384]
B5_BANDS = [
    (0, 248, 0, 0),
    (248, 496, 1, 4),
    (496, 512, 2, 4),
    (512, 744, 2, 20),
    (744, 992, 3, 4),
    (992, 1024, 4, 224),
]


def _emit_block5(nc, ps, src5, mlo, mhi, cax):
    """5 disjoint-band matmuls (+1 bank split) for one [128,1024] block.

    src5: 5 K-tiles with partition = contraction dim at offsets B5_OFF;
    each output band is written exactly once (start=True clears only
    written-bits, data of other bands persists for the evac read)."""
    for (lo, hi, t, clo) in B5_BANDS:
        nc.tensor.matmul(ps[:, lo:hi], lhsT=src5[t][:, mlo:mhi],
                         rhs=cax[:, clo:clo + (hi - lo)],
                         start=True, stop=True, skip_group_check=True)


def _emit_block4(nc, ps, src, mlo, mhi, cax, cbx):
    """4 matmuls per [128,1024] block: each interior K-tile's full 263-col
    contribution is ONE band (a cbx slice) that crosses the 512-col PSUM
    bank boundary - replaces the 6-MM main+corner split (1046 streamed
    cols vs 1560). Order matters: the two non-crossing MMs carry
    start=True and must clear their bank's has_written bits BEFORE the
    crossing MMs write into either bank (overlaps then accumulate)."""
    mm = nc.tensor.matmul
    kw = dict(skip_group_check=True)
    mm(ps[:, 0:260], lhsT=src[0][:, mlo:mhi], rhs=cax[:, 0:260],
       start=True, stop=False, **kw)
    mm(ps[:, 764:1024], lhsT=src[3][:, mlo:mhi], rhs=cbx[:, 256:516],
       start=True, stop=False, **kw)
    mm(ps[:, 252:515], lhsT=src[1][:, mlo:mhi], rhs=cbx[:, 256:519],
       start=False, stop=False, **kw)
    mm(ps[:, 508:771], lhsT=src[2][:, mlo:mhi], rhs=cbx[:, 256:519],
       start=False, stop=True, **kw)


def _build_program(reps=1, timing_mode=False, loop_n=None,
                   skip_in=False, skip_out=False, skip_compute=False):
    import concourse.bass as bass
    import concourse.mybir as mybir
    import concourse.tile as tile

    f32 = mybir.dt.float32
    dmm = {"f32r": mybir.dt.float32r, "f32": f32,
           "bf16": mybir.dt.bfloat16}[MM_DTYPE]
    dout = mybir.dt.bfloat16 if OUT_DTYPE == "bf16" else f32

    nc = bass.Bass("TRN2", target_bir_lowering=False, debug=False,
                   num_devices=N_CORES)
    if timing_mode:
        # same dataflow, but keep the big tensors device-internal so the
        # per-call wall isn't dominated by host<->device shipping
        x_d = nc.dram_tensor("x", [B_PER_CORE, H, W], dmm, kind="Internal")
        y_d = nc.dram_tensor("y", [B_PER_CORE, HO, WO], dout, kind="Internal")
        ydummy_d = nc.dram_tensor("ydummy", [1, 4], dmm, kind="ExternalOutput")
    else:
        x_d = nc.dram_tensor("x", [B_PER_CORE, H, W], dmm, kind="ExternalInput")
        y_d = nc.dram_tensor("y", [B_PER_CORE, HO, WO], dout, kind="ExternalOutput")
    c_d = nc.dram_tensor("c", [128, 1040], dmm, kind="ExternalInput")

    with tile.TileContext(nc) as tc:
        with tc.tile_pool(name="consts", bufs=1) as constp, \
             tc.tile_pool(name="xp", bufs=XP_BUFS) as xp, \
             tc.tile_pool(name="t1p", bufs=T1P_BUFS) as t1p, \
             tc.tile_pool(name="yp", bufs=YP_BUFS) as yp, \
             tc.tile_pool(name="psp", bufs=PSP_BUFS, space="PSUM") as psp:

            c_t = constp.tile([128, 1040], dmm)
            nc.scalar.dma_start(out=c_t[:], in_=c_d.ap())
            if timing_mode:
                nc.sync.dma_start(out=ydummy_d.ap(), in_=c_t[0:1, 0:4])
            cax = c_t[:, 0:520]
            cbx = c_t[:, 520:1040]

            copy_flip = [0]

            def evac(ps, out_tile):
                mode = EVAC_MODE
                i = copy_flip[0]
                copy_flip[0] += 1
                if mode == "ratio":
                    # ACT on a Bresenham pattern with density EVAC_RATIO
                    # (ACT is ~1.25x faster per element than DVE)
                    if int((i + 1) * EVAC_RATIO) > int(i * EVAC_RATIO):
                        nc.scalar.copy(out=out_tile[:], in_=ps[:])
                    else:
                        nc.vector.tensor_copy(out=out_tile[:], in_=ps[:])
                elif mode == "act":
                    nc.scalar.copy(out=out_tile[:], in_=ps[:])
                elif mode == "dve":
                    nc.vector.tensor_copy(out=out_tile[:], in_=ps[:])
                elif mode == "alt":
                    if i % 2 == 0:
                        nc.scalar.copy(out=out_tile[:], in_=ps[:])
                    else:
                        nc.vector.tensor_copy(out=out_tile[:], in_=ps[:])
                elif mode == "rot21":
                    if i % 3 < 2:
                        nc.scalar.copy(out=out_tile[:], in_=ps[:])
                    else:
                        nc.vector.tensor_copy(out=out_tile[:], in_=ps[:])
                elif mode == "banksplit":
                    # ACT bank0, DVE bank1 (different banks, concurrent)
                    nc.scalar.copy(out=out_tile[:, 0:512], in_=ps[:, 0:512])
                    nc.vector.tensor_copy(out=out_tile[:, 512:1024], in_=ps[:, 512:1024])
                else:
                    raise ValueError(mode)

            def evac_wide(ps, out_flat, n):
                # one ACT + one DVE copy per [128, n] PSUM region, split to
                # equalize engine time (ACT .833ns/col + 185ns fixed, DVE
                # 1.042ns/col + 125ns fixed); wide copies amortize the
                # per-instruction PSUM/SBUF access penalty
                a = (n * 556) // 1000 // 16 * 16
                nc.scalar.copy(out=out_flat[:, 0:a], in_=ps[:, 0:a])
                nc.vector.tensor_copy(out=out_flat[:, a:n], in_=ps[:, a:n])

            def body(refill_late=False):
                in_eng = {"scalar": nc.scalar, "gpsimd": nc.gpsimd,
                          "sync": nc.sync}[IN_ENG]
                if SCHEME in ("corner", "corner4", "hybrid"):
                    # single input DMA: one read burst instead of 4 cuts HBM
                    # read/write turnaround (measured 28.5 vs 34.8 mixed floor)
                    x_big = xp.tile([128, 2 * 4, W], dmm, tag="x", name="x_big")

                    def load_x():
                        in_eng.dma_start(
                            out=x_big[:],
                            in_=x_d.ap().rearrange("b (t p) w -> p (b t) w", p=128))

                    xts = [[x_big[:, 4 * b + t, :] for t in range(4)]
                           for b in range(B_PER_CORE)]
                else:
                    # 5 K-tiles per image at B5_OFF; slots 0-2 = the three
                    # stride-124 tiles (one DMA), slots 3-4 = the two aligned
                    # tiles (second DMA). 640 rows read per image (+25%).
                    x_big = xp.tile([128, B_PER_CORE, 5, W], dmm, tag="x",
                                    name="x_big")
                    xap = x_d.ap()
                    APc = type(xap)

                    def load_x():
                        # DMA APs balance only up to 3 dims, so one pair of
                        # DMAs per image: strided-124 tiles + aligned pair
                        for b in range(B_PER_CORE):
                            in_eng.dma_start(
                                out=x_big[:, b, 0:3, :],
                                in_=APc(xap.tensor, b * H * W + 122 * W,
                                        [[W, 128], [124 * W, 3], [1, W]]))
                            in_eng.dma_start(
                                out=x_big[:, b, 3:5, :],
                                in_=APc(xap.tensor, b * H * W,
                                        [[W, 128], [384 * W, 2], [1, W]]))

                    slot_of = [3, 0, 1, 2, 4]
                    xts = [[x_big[:, b, slot_of[t], :] for t in range(5)]
                           for b in range(B_PER_CORE)]

                if skip_in:
                    nc.gpsimd.memset(x_big[:].bitcast(f32), 0.0)
                elif not refill_late:
                    load_x()

                # stage 1 for BOTH images first: gives stage 2 a full
                # image's worth of PE runway behind the t1 evacuations,
                # keeping the PE gap-free (p-state ramp needs >3us of
                # continuous busy to reach 2.4GHz; gaps halve the clock)
                t1s = []
                n_t1 = 4 if SCHEME in ("corner", "corner4") else 5
                for b in range(B_PER_CORE):
                    t1 = []
                    for m in range(n_t1):
                        t1m = t1p.tile([128, 1024], dmm, tag="t1",
                                       name=f"t1_{b}_{m}")
                        if not skip_compute:
                            ps = psp.tile([128, 1024], f32, tag="ps",
                                          name=f"ps1_{b}_{m}")
                            if SCHEME == "banded5":
                                _emit_block5(nc, ps, xts[b], B5_OFF[m],
                                             B5_OFF[m] + 128, cax)
                            elif SCHEME == "corner4":
                                _emit_block4(nc, ps, xts[b], 128 * m,
                                             128 * (m + 1), cax, cbx)
                            else:
                                mlo = 128 * m if SCHEME == "corner" else B5_OFF[m]
                                _emit_block(nc, ps, xts[b], mlo, mlo + 128,
                                            cax, cbx, dmm)
                            evac(ps, t1m)
                        t1.append(t1m)
                    t1s.append(t1)

                if refill_late and not skip_in:
                    # timing loop only: refill this body's x buffer for the
                    # NEXT For_i iteration now that stage 1 consumed it; the
                    # trigger sits mid-stream on ACT so the transfer lands
                    # ~1.5 reps before its consumer instead of arriving at
                    # the rep boundary (loop data is identical every rep)
                    load_x()

                for b in range(B_PER_CORE):
                    for rp in range(8 // Y_GROUP):
                        y_pair = yp.tile([128, Y_GROUP, 1024], dout, tag="y",
                                         name=f"y_{b}_{rp}")
                        for j in range(Y_GROUP):
                            r = Y_GROUP * rp + j
                            if not skip_compute:
                                ps = psp.tile([128, 1024], f32, tag="ps",
                                              name=f"ps2_{b}_{r}")
                                if SCHEME == "corner":
                                    _emit_block(nc, ps, t1s[b], 128 * r,
                                                128 * (r + 1), cax, cbx, dmm)
                                elif SCHEME == "corner4":
                                    _emit_block4(nc, ps, t1s[b], 128 * r,
                                                 128 * (r + 1), cax, cbx)
                                else:  # banded5 and hybrid: banded stage 2
                                    _emit_block5(nc, ps, t1s[b], 128 * r,
                                                 128 * (r + 1), cax)
                                evac(ps, y_pair[:, j, :])
                            else:
                                nc.gpsimd.memset(y_pair[:, j, :], 0.0)
                        if not skip_out:
                            eng = nc.sync if (OUT_RING == "sync" or rp % 2 == 0) else nc.scalar
                            eng.dma_start(
                                out=y_d.ap()[b].rearrange(
                                    "(r p) c -> p r c", p=128)[
                                        :, Y_GROUP * rp:Y_GROUP * (rp + 1), :],
                                in_=y_pair[:])

            if loop_n is not None:
                assert loop_n % UNROLL == 0
                with tc.For_i(0, loop_n // UNROLL, 1, staggered_reset=STAGGERED):
                    for _ in range(UNROLL):
                        body(refill_late=True)
            else:
                for _ in range(reps):
                    body()

    _split_multiwaits(nc, mybir)
    return nc


def _host_dt():
    if MM_DTYPE == "bf16":
        import ml_dtypes
        return np.dtype(ml_dtypes.bfloat16)
    return np.dtype(np.float32)


def _get_program():
    if "nc" not in _CACHE:
        _CACHE["nc"] = _build_program()
        _CACHE["c"] = np.ascontiguousarray(_consts().astype(_host_dt()))
    return _CACHE["nc"], _CACHE["c"]


def kernel(image_batch: np.ndarray) -> np.ndarray:
    from concourse.bass_utils import run_bass_kernel_spmd

    nc, c = _get_program()
    x = np.ascontiguousarray(
        np.asarray(image_batch, dtype=np.float32).reshape(16, H, W)
        .astype(_host_dt()))
    in_maps = [
        {"x": x[B_PER_CORE * k:B_PER_CORE * (k + 1)], "c": c}
        for k in range(N_CORES)
    ]
    res = run_bass_kernel_spmd(nc, in_maps, core_ids=list(range(N_CORES)))
    out = np.concatenate([np.asarray(r["y"]) for r in res.results], axis=0)
    return out.astype(np.float32).reshape(16, HO, WO, 1)



# revision 48
# speedup vs baseline: 1.0554x; 1.0554x over previous
"""Bior 2x upsampling (zero-interleave + separable 9-tap filter) on 8 TRN2 cores.

Math: y[n] = sum_m h[n+4-2m] x[m] along each spatial axis (SAME zero padding).
Both separable stages are banded matmuls on the TensorEngine:

  stage 1: T1[w, nh] = sum_h X[h, w]  * A[nh, h]   (lhsT = X,  K = h)
  stage 2: Y[nh, nw] = sum_w T1[w, nh] * A[nw, w]  (lhsT = T1, K = w)

with A[n, m] = h[n+4-2m]. Because A is shift invariant, every matmul's rhs
is a column-slice of constant matrices CAx[i,j] = h[j+4-2i] /
CBx[i,j] = h[j-256-2i] ([128, 520] each, shipped once).

Everything is bf16 except PSUM accumulation (f32): input shipped bf16
(1 MiB/core), output stored bf16 (4 MiB/core) and upcast to f32 on the
host - halves HBM traffic vs f32 at ~7.6e-3 rel err (tolerance 2e-2).
bf16 matmuls also avoid the f32r N<256 cycle penalty.

Default SCHEME "corner4": K-tiles 128-aligned in both stages, so ONE
input DMA burst loads x (avoids HBM read/write turnaround); each
128x1024 block is FOUR matmuls - each interior K-tile's full 263-col
contribution is one contiguous cbx band that CROSSES the 512-col PSUM
bank boundary (legal on this toolchain). 1046 streamed cols/block vs
1560 for the classic 6-MM main+corner split ("corner"), cutting PE busy
15.6->10.5us/rep. Ordering constraint: the two non-crossing MMs carry
start=True and clear each bank's has_written bits FIRST; the crossing
MMs then accumulate on overlaps / overwrite elsewhere.
Alternatives kept for reference: "corner" (6 uniform MMs), and
"banded5"/"hybrid" (K-tiles at B5_OFF with 4-row overlaps so every band
is disjoint - but +2 evac tiles/rep and 4 input DMAs make them slower;
evac, not PE, is the tighter resource).

Pipelining (the difference between 37us and ~16us at identical work):
 - PE p-state: the tensor engine only reaches 2.4GHz after ~3us of
   gap-free execution (else 1.2GHz). All structure below serves PE
   density: stage 1 of BOTH images runs before any stage 2 (evac runway),
   4 narrow [128,1024] PSUM bufs give 3-group rotation slack (~2us).
 - PSUM evacuation (PSUM->SBUF copies; DMA cannot read PSUM) is the
   second-largest engine cost (~26624 cols/rep). Whole-tile copies
   alternate ACT/DVE on a Bresenham pattern (EVAC_RATIO on ACT, which is
   ~1.25x faster/elem); per-tile splitting costs more in fixed
   PSUM-access overhead (~185ns/instr) than it saves.
 - input DMA triggers ride the otherwise-idle SP(sync) ring - on ACT
   they queue behind a whole rep of evacs and serialize the input.
 - timing loop: tile pools only rotate per tile() call at trace time, so
   UNROLL reps are emitted per For_i body for cross-rep double-buffering,
   and each body's x buffer is refilled MID-body (loop data is identical
   every rep) giving ~1.5 reps of prefetch distance.

Sharding: pure data parallel, 2 images per core across 8 cores.
Engine floors per rep (2 images): PE 10.5us, ACT/DVE evac ~14/12us at
EVAC_RATIO 0.58, DMA 13.7-15.3us (5.25 MiB at ~360-400GB/s) ->
measured ~15.2-16.6us (was 37.4us baseline).
"""

import numpy as np

H_TILDE = np.array([0.03782845550699535, -0.02384946501937986, -0.1106244044184226,
                    0.3774028556126536, 0.8526986790094022, 0.3774028556126537,
                    -0.1106244044184226, -0.02384946501937986, 0.03782845550699535],
                   dtype=np.float32)

B_PER_CORE = 2
N_CORES = 8
H = W = 512
HO = WO = 1024

# "bf16" (2x PE rate + half DMA bytes, ~5e-3 rel err), "f32r" (~2e-4),
# or "f32" (~1e-6, 8x slower matmuls)
MM_DTYPE = "bf16"
# y stored in HBM as bf16 (half the output write traffic), upcast on host
OUT_DTYPE = "bf16"
EVAC_MODE = "ratio"
# "corner": aligned 128-row K-tiles + full-width corner MMs (6 MMs, 1560
# cols per 128x1024 block). "banded5": K-tiles at stride 124 (4-row
# overlap, re-reads 16 rows/image of x) so each K-tile contributes one
# disjoint output band - 1024 cols per block, no corners. "hybrid":
# corner structure for stage 1 (keeps the single aligned input DMA) but
# its 5 M-slices sit at B5_OFF so stage 2 runs banded - 1024 cols/block.
# corner measured fastest: banded/hybrid trade PE cols for 2 extra evac
# tiles/rep, and evac is the tighter resource.
SCHEME = "corner4"
# fraction of evac tiles copied by ACT (rest DVE); with corner4's light
# PE the evac engines bind, and ACT-heavy measures best
EVAC_RATIO = 0.58
OUT_RING = "sync"
IN_ENG = "sync"
STAGGERED = False
MM_ORDER = "banks"
# reps unrolled inside the For_i timing body: tile pools only rotate per
# tile() call at trace time, so cross-iteration double-buffering (x
# prefetch, y drain) needs >=2 reps in the traced body
UNROLL = 4
PSP_BUFS = 4
XP_BUFS = 2
# 16 gives stage-1 evacs 2 reps of WAR distance from stage-2 readers
T1P_BUFS = 16
YP_BUFS = 4
# output rows per DMA: 4 -> four 1MiB writes/rep instead of eight 512KiB
# (fewer DMA instruction overheads + write-burst turnarounds, ~1.2us)
Y_GROUP = 4

_CACHE = {}


def _consts():
    """One [128, 1040] f32 constant: CAx | CBx (each [128, 520]).

    CAx[i, j] = h[j + 4 - 2i], CBx[i, j] = h[j - 256 - 2i]. Slices:
      main  rhs aligned at +0   : cax[:, 0:260]   /  cbx[:, 256:516]
      corner rhs (same N=260)   : cax[:, 256:516] /  cbx[:, 0:260]
    """
    h = H_TILDE
    cax = np.zeros((128, 520), dtype=np.float32)
    cbx = np.zeros((128, 520), dtype=np.float32)
    for i in range(128):
        for j in range(520):
            k = j + 4 - 2 * i
            if 0 <= k <= 8:
                cax[i, j] = h[k]
            k = j - 256 - 2 * i
            if 0 <= k <= 8:
                cbx[i, j] = h[k]
    return np.concatenate([cax, cbx], axis=1)


def _split_multiwaits(nc, mybir):
    """walrus here encodes at most ONE sem-wait per instruction; hoist extras
    onto preceding same-engine nops (sequencer order => identical semantics)."""
    ctr = 0
    for fn in nc.m.functions:
        for bb in fn.blocks:
            out, changed = [], False
            for ins in bb.instructions:
                si = ins.sync_info
                if si is not None and len(si.on_wait) > 1:
                    waits = list(si.on_wait)
                    for w in waits[:-1]:
                        ctr += 1
                        nop = mybir.InstNoOp(name=f"wsplit-{ctr}", ins=[], outs=[])
                        nop.engine = ins.engine
                        nop.sync_info = mybir.SyncInfo(on_wait=[w], on_update=[])
                        out.append(nop)
                    si.on_wait = [waits[-1]]
                    changed = True
                out.append(ins)
            if changed:
                bb.instructions = out
    return ctr


def _emit_block(nc, ps, src, mlo, mhi, cax, cbx, f32r, MM_ORDER=None):
    if MM_ORDER is None:
        MM_ORDER = globals()["MM_ORDER"]
    """Emit the 6 uniform [K=128, M=128, N=260] matmuls for one block.

    ps: PSUM [128, 1024]; src: 4 source tiles (partitions = contraction dim);
    mlo:mhi: the 128-wide free-dim slice of the source tiles forming M.
    Corners are full-shape MMs whose rhs is mostly zeros (uniform shape
    keeps the PE pipeline dense; tiny-N MMs measured ~600ns each)."""
    mm = nc.tensor.matmul
    kw = dict(skip_group_check=True)
    if MM_ORDER == "banks":
        mm(ps[:, 0:260], lhsT=src[0][:, mlo:mhi], rhs=cax[:, 0:260],
           start=True, stop=False, **kw)
        mm(ps[:, 252:512], lhsT=src[1][:, mlo:mhi], rhs=cbx[:, 256:516],
           start=False, stop=False, **kw)
        mm(ps[:, 252:512], lhsT=src[2][:, mlo:mhi], rhs=cbx[:, 0:260],
           start=False, stop=False, **kw)
        mm(ps[:, 512:772], lhsT=src[2][:, mlo:mhi], rhs=cax[:, 0:260],
           start=True, stop=False, **kw)
        mm(ps[:, 512:772], lhsT=src[1][:, mlo:mhi], rhs=cax[:, 256:516],
           start=False, stop=False, **kw)
        mm(ps[:, 764:1024], lhsT=src[3][:, mlo:mhi], rhs=cbx[:, 256:516],
           start=False, stop=True, **kw)
    else:  # "paired": same-lhsT MMs adjacent; bank1's first writer is the
           # tile1 corner (start=True overwrites with zeros+corner, then
           # tile2 main accumulates) — identical math via has_written rules
        mm(ps[:, 0:260], lhsT=src[0][:, mlo:mhi], rhs=cax[:, 0:260],
           start=True, stop=False, **kw)
        mm(ps[:, 252:512], lhsT=src[1][:, mlo:mhi], rhs=cbx[:, 256:516],
           start=False, stop=False, **kw)
        mm(ps[:, 512:772], lhsT=src[1][:, mlo:mhi], rhs=cax[:, 256:516],
           start=True, stop=False, **kw)
        mm(ps[:, 252:512], lhsT=src[2][:, mlo:mhi], rhs=cbx[:, 0:260],
           start=False, stop=False, **kw)
        mm(ps[:, 512:772], lhsT=src[2][:, mlo:mhi], rhs=cax[:, 0:260],
           start=False, stop=False, **kw)
        mm(ps[:, 764:1024], lhsT=src[3][:, mlo:mhi], rhs=cbx[:, 256:516],
           start=False, stop=True, **kw)


# banded5: K-tile row offsets (4-row overlap at stride 124; tiles 0 and 4
# aligned so all reads stay in [0, 512)) and the disjoint output band each
# K-tile produces: (out_lo, out_hi, ktile, rhs col offset into cax).
# A band crossing the 512-col PSUM bank boundary is split at 512.
B5_OFF = [0, 122, 246, 370, 384]
B5_BANDS = [
    (0, 248, 0, 0),
    (248, 496, 1, 4),
    (496, 512, 2, 4),
    (512, 744, 2, 20),
    (744, 992, 3, 4),
    (992, 1024, 4, 224),
]


def _emit_block5(nc, ps, src5, mlo, mhi, cax):
    """5 disjoint-band matmuls (+1 bank split) for one [128,1024] block.

    src5: 5 K-tiles with partition = contraction dim at offsets B5_OFF;
    each output band is written exactly once (start=True clears only
    written-bits, data of other bands persists for the evac read)."""
    for (lo, hi, t, clo) in B5_BANDS:
        nc.tensor.matmul(ps[:, lo:hi], lhsT=src5[t][:, mlo:mhi],
                         rhs=cax[:, clo:clo + (hi - lo)],
                         start=True, stop=True, skip_group_check=True)


def _emit_block4(nc, ps, src, mlo, mhi, cax, cbx):
    """4 matmuls per [128,1024] block: each interior K-tile's full 263-col
    contribution is ONE band (a cbx slice) that crosses the 512-col PSUM
    bank boundary - replaces the 6-MM main+corner split (1046 streamed
    cols vs 1560). Order matters: the two non-crossing MMs carry
    start=True and must clear their bank's has_written bits BEFORE the
    crossing MMs write into either bank (overlaps then accumulate)."""
    mm = nc.tensor.matmul
    kw = dict(skip_group_check=True)
    mm(ps[:, 0:260], lhsT=src[0][:, mlo:mhi], rhs=cax[:, 0:260],
       start=True, stop=False, **kw)
    mm(ps[:, 764:1024], lhsT=src[3][:, mlo:mhi], rhs=cbx[:, 256:516],
       start=True, stop=False, **kw)
    mm(ps[:, 252:515], lhsT=src[1][:, mlo:mhi], rhs=cbx[:, 256:519],
       start=False, stop=False, **kw)
    mm(ps[:, 508:771], lhsT=src[2][:, mlo:mhi], rhs=cbx[:, 256:519],
       start=False, stop=True, **kw)


def _build_program(reps=1, timing_mode=False, loop_n=None,
                   skip_in=False, skip_out=False, skip_compute=False):
    import concourse.bass as bass
    import concourse.mybir as mybir
    import concourse.tile as tile

    f32 = mybir.dt.float32
    dmm = {"f32r": mybir.dt.float32r, "f32": f32,
           "bf16": mybir.dt.bfloat16}[MM_DTYPE]
    dout = mybir.dt.bfloat16 if OUT_DTYPE == "bf16" else f32

    nc = bass.Bass("TRN2", target_bir_lowering=False, debug=False,
                   num_devices=N_CORES)
    if timing_mode:
        # same dataflow, but keep the big tensors device-internal so the
        # per-call wall isn't dominated by host<->device shipping
        x_d = nc.dram_tensor("x", [B_PER_CORE, H, W], dmm, kind="Internal")
        y_d = nc.dram_tensor("y", [B_PER_CORE, HO, WO], dout, kind="Internal")
        ydummy_d = nc.dram_tensor("ydummy", [1, 4], dmm, kind="ExternalOutput")
    else:
        x_d = nc.dram_tensor("x", [B_PER_CORE, H, W], dmm, kind="ExternalInput")
        y_d = nc.dram_tensor("y", [B_PER_CORE, HO, WO], dout, kind="ExternalOutput")
    c_d = nc.dram_tensor("c", [128, 1040], dmm, kind="ExternalInput")

    with tile.TileContext(nc) as tc:
        with tc.tile_pool(name="consts", bufs=1) as constp, \
             tc.tile_pool(name="xp", bufs=XP_BUFS) as xp, \
             tc.tile_pool(name="t1p", bufs=T1P_BUFS) as t1p, \
             tc.tile_pool(name="yp", bufs=YP_BUFS) as yp, \
             tc.tile_pool(name="psp", bufs=PSP_BUFS, space="PSUM") as psp:

            c_t = constp.tile([128, 1040], dmm)
            nc.scalar.dma_start(out=c_t[:], in_=c_d.ap())
            if timing_mode:
                nc.sync.dma_start(out=ydummy_d.ap(), in_=c_t[0:1, 0:4])
            cax = c_t[:, 0:520]
            cbx = c_t[:, 520:1040]

            copy_flip = [0]

            def evac(ps, out_tile):
                mode = EVAC_MODE
                i = copy_flip[0]
                copy_flip[0] += 1
                if mode == "ratio":
                    # ACT on a Bresenham pattern with density EVAC_RATIO
                    # (ACT is ~1.25x faster per element than DVE)
                    if int((i + 1) * EVAC_RATIO) > int(i * EVAC_RATIO):
                        nc.scalar.copy(out=out_tile[:], in_=ps[:])
                    else:
                        nc.vector.tensor_copy(out=out_tile[:], in_=ps[:])
                elif mode == "act":
                    nc.scalar.copy(out=out_tile[:], in_=ps[:])
                elif mode == "dve":
                    nc.vector.tensor_copy(out=out_tile[:], in_=ps[:])
                elif mode == "alt":
                    if i % 2 == 0:
                        nc.scalar.copy(out=out_tile[:], in_=ps[:])
                    else:
                        nc.vector.tensor_copy(out=out_tile[:], in_=ps[:])
                elif mode == "rot21":
                    if i % 3 < 2:
                        nc.scalar.copy(out=out_tile[:], in_=ps[:])
                    else:
                        nc.vector.tensor_copy(out=out_tile[:], in_=ps[:])
                elif mode == "banksplit":
                    # ACT bank0, DVE bank1 (different banks, concurrent)
                    nc.scalar.copy(out=out_tile[:, 0:512], in_=ps[:, 0:512])
                    nc.vector.tensor_copy(out=out_tile[:, 512:1024], in_=ps[:, 512:1024])
                else:
                    raise ValueError(mode)

            def evac_wide(ps, out_flat, n):
                # one ACT + one DVE copy per [128, n] PSUM region, split to
                # equalize engine time (ACT .833ns/col + 185ns fixed, DVE
                # 1.042ns/col + 125ns fixed); wide copies amortize the
                # per-instruction PSUM/SBUF access penalty
                a = (n * 556) // 1000 // 16 * 16
                nc.scalar.copy(out=out_flat[:, 0:a], in_=ps[:, 0:a])
                nc.vector.tensor_copy(out=out_flat[:, a:n], in_=ps[:, a:n])

            def body(refill_late=False):
                in_eng = {"scalar": nc.scalar, "gpsimd": nc.gpsimd,
                          "sync": nc.sync}[IN_ENG]
                if SCHEME in ("corner", "corner4", "hybrid"):
                    # single input DMA: one read burst instead of 4 cuts HBM
                    # read/write turnaround (measured 28.5 vs 34.8 mixed floor)
                    x_big = xp.tile([128, 2 * 4, W], dmm, tag="x", name="x_big")

                    def load_x():
                        in_eng.dma_start(
                            out=x_big[:],
                            in_=x_d.ap().rearrange("b (t p) w -> p (b t) w", p=128))

                    xts = [[x_big[:, 4 * b + t, :] for t in range(4)]
                           for b in range(B_PER_CORE)]
                else:
                    # 5 K-tiles per image at B5_OFF; slots 0-2 = the three
                    # stride-124 tiles (one DMA), slots 3-4 = the two aligned
                    # tiles (second DMA). 640 rows read per image (+25%).
                    x_big = xp.tile([128, B_PER_CORE, 5, W], dmm, tag="x",
                                    name="x_big")
                    xap = x_d.ap()
                    APc = type(xap)

                    def load_x():
                        # DMA APs balance only up to 3 dims, so one pair of
                        # DMAs per image: strided-124 tiles + aligned pair
                        for b in range(B_PER_CORE):
                            in_eng.dma_start(
                                out=x_big[:, b, 0:3, :],
                                in_=APc(xap.tensor, b * H * W + 122 * W,
                                        [[W, 128], [124 * W, 3], [1, W]]))
                            in_eng.dma_start(
                                out=x_big[:, b, 3:5, :],
                                in_=APc(xap.tensor, b * H * W,
                                        [[W, 128], [384 * W, 2], [1, W]]))

                    slot_of = [3, 0, 1, 2, 4]
                    xts = [[x_big[:, b, slot_of[t], :] for t in range(5)]
                           for b in range(B_PER_CORE)]

                if skip_in:
                    nc.gpsimd.memset(x_big[:].bitcast(f32), 0.0)
                elif not refill_late:
                    load_x()

                # stage 1 for BOTH images first: gives stage 2 a full
                # image's worth of PE runway behind the t1 evacuations,
                # keeping the PE gap-free (p-state ramp needs >3us of
                # continuous busy to reach 2.4GHz; gaps halve the clock)
                t1s = []
                n_t1 = 4 if SCHEME in ("corner", "corner4") else 5
                for b in range(B_PER_CORE):
                    t1 = []
                    for m in range(n_t1):
                        t1m = t1p.tile([128, 1024], dmm, tag="t1",
                                       name=f"t1_{b}_{m}")
                        if not skip_compute:
                            ps = psp.tile([128, 1024], f32, tag="ps",
                                          name=f"ps1_{b}_{m}")
                            if SCHEME == "banded5":
                                _emit_block5(nc, ps, xts[b], B5_OFF[m],
                                             B5_OFF[m] + 128, cax)
                            elif SCHEME == "corner4":
                                _emit_block4(nc, ps, xts[b], 128 * m,
                                             128 * (m + 1), cax, cbx)
                            else:
                                mlo = 128 * m if SCHEME == "corner" else B5_OFF[m]
                                _emit_block(nc, ps, xts[b], mlo, mlo + 128,
                                            cax, cbx, dmm)
                            evac(ps, t1m)
                        t1.append(t1m)
                    t1s.append(t1)

                if refill_late and not skip_in:
                    # timing loop only: refill this body's x buffer for the
                    # NEXT For_i iteration now that stage 1 consumed it; the
                    # trigger sits mid-stream on ACT so the transfer lands
                    # ~1.5 reps before its consumer instead of arriving at
                    # the rep boundary (loop data is identical every rep)
                    load_x()

                for b in range(B_PER_CORE):
                    for rp in range(8 // Y_GROUP):
                        y_pair = yp.tile([128, Y_GROUP, 1024], dout, tag="y",
                                         name=f"y_{b}_{rp}")
                        for j in range(Y_GROUP):
                            r = Y_GROUP * rp + j
                            if not skip_compute:
                                ps = psp.tile([128, 1024], f32, tag="ps",
                                              name=f"ps2_{b}_{r}")
                                if SCHEME == "corner":
                                    _emit_block(nc, ps, t1s[b], 128 * r,
                                                128 * (r + 1), cax, cbx, dmm)
                                elif SCHEME == "corner4":
                                    _emit_block4(nc, ps, t1s[b], 128 * r,
                                                 128 * (r + 1), cax, cbx)
                                else:  # banded5 and hybrid: banded stage 2
                                    _emit_block5(nc, ps, t1s[b], 128 * r,
                                                 128 * (r + 1), cax)
                                evac(ps, y_pair[:, j, :])
                            else:
                                nc.gpsimd.memset(y_pair[:, j, :], 0.0)
                        if not skip_out:
                            eng = nc.sync if (OUT_RING == "sync" or rp % 2 == 0) else nc.scalar
                            eng.dma_start(
                                out=y_d.ap()[b].rearrange(
                                    "(r p) c -> p r c", p=128)[
                                        :, Y_GROUP * rp:Y_GROUP * (rp + 1), :],
                                in_=y_pair[:])

            if loop_n is not None:
                assert loop_n % UNROLL == 0
                with tc.For_i(0, loop_n // UNROLL, 1, staggered_reset=STAGGERED):
                    for _ in range(UNROLL):
                        body(refill_late=True)
            else:
                for _ in range(reps):
                    body()

    _split_multiwaits(nc, mybir)
    return nc


def _host_dt():
    if MM_DTYPE == "bf16":
        import ml_dtypes
        return np.dtype(ml_dtypes.bfloat16)
    return np.dtype(np.float32)


def _get_program():
    if "nc" not in _CACHE:
        _CACHE["nc"] = _build_program()
        _CACHE["c"] = np.ascontiguousarray(_consts().astype(_host_dt()))
    return _CACHE["nc"], _CACHE["c"]


def kernel(image_batch: np.ndarray) -> np.ndarray:
    from concourse.bass_utils import run_bass_kernel_spmd

    nc, c = _get_program()
    x = np.ascontiguousarray(
        np.asarray(image_batch, dtype=np.float32).reshape(16, H, W)
        .astype(_host_dt()))
    in_maps = [
        {"x": x[B_PER_CORE * k:B_PER_CORE * (k + 1)], "c": c}
        for k in range(N_CORES)
    ]
    res = run_bass_kernel_spmd(nc, in_maps, core_ids=list(range(N_CORES)))
    out = np.concatenate([np.asarray(r["y"]) for r in res.results], axis=0)
    return out.astype(np.float32).reshape(16, HO, WO, 1)

